# revision 43
# baseline (speedup 1.0000x reference)
"""Single-head attention (B=4, S=4096, E=1024, D=64) on 8 TRN2 NeuronCores.

Sharding: data-parallel over (batch, query-half): core c handles batch
b = c // 2 and query rows [h*2048, (h+1)*2048) with h = c % 2. Each core
computes Q for its own 2048 rows and K/V for the full 4096 rows of its batch
(inputs are shipped host-pretransposed per half, so no duplicated DMA).

Per-core dataflow (TensorE matmuls in bf16 — fp32/fp32r matmuls run the PE
at half clock; fp32 accumulation in PSUM). Projections pack TWO weight
matrices into one 128-wide stationary operand:
  qk [128, 2048] = [K^T_own; Q^T_own]     (pass A, lhsT = [WkT | WqT])
  kv [128, 2048] = [K^T_oth; V^T_oth]     (pass B, lhsT = [WkT | WvT])
  vt [65, 2048]  = V^T_own + ones row      (pass C, lhsT = WvT)
Q^T / V^T_oth are then shifted to base partition 0 by SBUF-to-SBUF DMAs
(matmul operands must share a base partition).
  scores^T[k, q] = K^T.T @ Q^T -> exp -> P bf16
  attn^T[65, q] += V_aug.T @ P   (row 64 accumulates softmax denominators)
  output = attn^T with denominators; host transposes + normalizes.

The exp is split across two engines so ScalarE (1 elem/cycle/lane at
1.2 GHz, ~1.15us per [128,1024] tile) stops pacing the pipeline: 2/3 of
k-tiles get the exact ACT exp on ScalarE; every third tile is computed on
VectorE with a one-instruction Schraudolph bit-trick: i16 = rne(x*A + B)
reinterpreted as bf16 approximates exp(SCALE*x) (piecewise-linear mantissa
chord, ~1.8% log-error sawtooth whose mean bias cancels in the softmax
numerator/denominator ratio; applied to 20/64 of the weights it adds
~0.6% output rel err). The two engines use SEPARATE P-tile pools — a
shared pool serializes them via buffer-reuse ordering.

The attention runs as TWO passes over q (1024 columns each): the attn
accumulator then fits 2 PSUM banks, freeing a third scores slot (PSUM slot
contention paced the single-pass version), and pass 0's output ships
mid-kernel.

The HAM duty controller halves the PE clock after ~2.5us of PE idleness
and takes 5-13us at half clock to re-grant full duty, so the kernel keeps
the PE streaming: junk-fed 512-col warm-up matmuls run from the instant
the PE preamble ends until the first input DMAs land (gated only on a
VectorE memset, not on make_identity's gpsimd iota), and junk fillers
bridge the group-2 DMA wait at pass-0 iters 4-5. Input DMA is issued in
deadline order (wt + own q-cols 0:1024, then own 1024:2048, then the
other half) across the sync/scalar/gpsimd queues; the pass-0 side-slot
schedule (projection lumps A2/C2/A3/C3, then B0-B3) tracks the measured
arrival of those groups.
"""

import numpy as np

B, S, E, D = 4, 4096, 1024, 64
HALF = S // 2
N_CORES = 8
SCALE = 1.0 / np.sqrt(D)

NE = E // 128  # 8 e-tiles
NKT = S // 128  # 32 k-tiles
N_WARM = 20  # 512-col PE warm-up matmuls covering the preamble + DMA wait

# Schraudolph exp-approx constants (bf16 bit pattern via int16):
#   i16 = round(x * A16 + B16); bitcast bf16 ~= exp(SCALE * x)
LOG2E = 1.4426950408889634
A16 = SCALE * 128.0 * LOG2E
B16 = 127.0 * 128.0 - 7.3


def _exp_eng(k):
    """Which engine computes exp for k-tile `k` (0..NKT-1) of a pass.

    'S' = ScalarE exact ACT exp; 'V' = VectorE Schraudolph. (GpSimd
    cannot read PSUM, so it can't help here.) 22 exact + 10 approx per
    pass: the approx noise (~1.8% per weight) lands on 20/64 of the
    softmax weights -> ~0.6% output rel err.
    """
    return "V" if k % 3 == 2 else "S"

_CACHE = {}


def _build():
    if "nc" in _CACHE:
        return _CACHE["nc"]

    from contextlib import ExitStack

    import concourse.bacc as bacc
    import concourse.tile as tile
    from concourse import mybir
    from concourse.masks import make_identity

    FP32 = mybir.dt.float32
    BF16 = mybir.dt.bfloat16
    I16 = mybir.dt.int16
    Exp = mybir.ActivationFunctionType.Exp
    Mult = mybir.AluOpType.mult
    Add = mybir.AluOpType.add

    nc = bacc.Bacc(
        "TRN2", target_bir_lowering=False, debug=False, num_devices=N_CORES
    )

    xt_q_d = nc.dram_tensor("xt_q", [E, HALF], BF16, kind="ExternalInput").ap()
    xt_o_d = nc.dram_tensor("xt_o", [E, HALF], BF16, kind="ExternalInput").ap()
    wt_d = nc.dram_tensor("wt", [E, 320], BF16, kind="ExternalInput").ap()
    out_d = nc.dram_tensor("out", [D + 1, HALF], FP32, kind="ExternalOutput").ap()

    with tile.TileContext(nc) as tc, ExitStack() as ctx:
        const = ctx.enter_context(tc.tile_pool(name="const", bufs=1))
        big = ctx.enter_context(tc.tile_pool(name="big", bufs=1))
        # separate P pools per exp engine: a shared pool serializes the
        # scalar and vector exps against each other via slot-reuse ordering
        pps = ctx.enter_context(tc.tile_pool(name="pps", bufs=4))
        ppv = ctx.enter_context(tc.tile_pool(name="ppv", bufs=3))
        psA = ctx.enter_context(tc.tile_pool(name="psA", bufs=3, space="PSUM"))
        psB = ctx.enter_context(tc.tile_pool(name="psB", bufs=1, space="PSUM"))

        identB = const.tile([128, 128], BF16)
        junk = const.tile([128, 512], BF16)
        # memset on vector: ready right at the end of vector's preamble, so
        # the PE warm-up can start ~2.5us before make_identity's gpsimd
        # iota would allow.
        nc.vector.memset(junk[:, :], 0.0)
        make_identity(nc, identB)

        xt = big.tile([128, NE, S], BF16)  # x^T; cols [0, HALF) = own q-rows
        wt = big.tile([128, NE, 320], BF16)  # [WkT|WqT | WkT|WvT | WvT]
        qk = big.tile([128, HALF], BF16)  # rows 0-63 K^T own, 64-127 Q^T own
        kv = big.tile([128, HALF], BF16)  # rows 0-63 K^T oth, 64-127 V^T oth
        qts = big.tile([64, HALF], BF16)  # Q^T shifted to base partition 0
        vto = big.tile([64, HALF], BF16)  # V^T other shifted to base part. 0
        vt = big.tile([65, HALF], BF16)  # V^T own; row 64 = ones
        vn = big.tile([128, NKT, D + 1], BF16)  # V natural + ones column
        att_sb = big.tile([65, HALF], FP32)  # attn^T + denominator row

        # --- PE warm-up: the HAM duty controller halves the PE clock after
        # ~2.5us of idleness and takes 5-13us at half clock to restore full
        # duty, so keep the PE streaming junk matmuls from the instant its
        # preamble ends until the first input DMAs land.
        warm = psA.tile([128, 1024], FP32, tag="ps")
        for _ in range(N_WARM):
            nc.tensor.matmul(
                out=warm[0:128, 0:512],
                lhsT=junk[:, 0:128],
                rhs=junk[:, :],
                start=True,
                stop=True,
            )

        # --- input DMAs. Descriptor generation costs ~0.6us PER dma_start
        # on the issuing engine's sequencer, so use few, fat instructions
        # and spread them over sync/scalar/gpsimd so everything is issued
        # (= streaming) within the first few us. Priority: wt + own cols
        # 0:512 (gates proj-A half 0), own 512:1024, own 1024:2048 (A2/A3
        # lumps, ~iter 5+), then other half (B lumps, ~iter 13+).
        nc.sync.dma_start(out=wt[:, :, :], in_=wt_d.rearrange("(t p) d -> p t d", p=128))
        # cols 0:1024 (gates the first exp): spread across all three
        # DMA-capable engines; gpsimd's SWDGE has multiple queues.
        first_engs = [nc.gpsimd, nc.sync, nc.gpsimd, nc.scalar,
                      nc.gpsimd, nc.sync, nc.gpsimd, nc.scalar]
        for et in range(NE):
            first_engs[et].dma_start(
                out=xt[:, et, 0:1024],
                in_=xt_q_d[et * 128 : (et + 1) * 128, 0:1024],
            )
        for et in range(NE):
            nc.gpsimd.dma_start(
                out=xt[:, et, 1024:2048],
                in_=xt_q_d[et * 128 : (et + 1) * 128, 1024:2048],
            )
        for et in range(NE):
            eng = [nc.gpsimd, nc.sync][et % 2]
            eng.dma_start(
                out=xt[:, et, HALF:S],
                in_=xt_o_d[et * 128 : (et + 1) * 128, :],
            )

        nc.vector.memset(vt[64:65, :], 1.0)

        # one packed projection half-chunk of 512 cols. The PSUM->SBUF copy
        # engine is selectable: VectorE's in-order queue (exps + transpose
        # copies) backs up in pass 0, so the A/B side-lump copies go to
        # ScalarE instead (its `copy` shares the exp activation-table set,
        # so no table-switch cost).
        def proj_half(w0, wm, dst, src_x0, d0, on_scalar=False):
            acc = psA.tile([128, 1024], FP32, tag="ps")
            for et in range(NE):
                nc.tensor.matmul(
                    out=acc[0:wm, 0:512],
                    lhsT=wt[:, et, w0 : w0 + wm],
                    rhs=xt[:, et, src_x0 : src_x0 + 512],
                    start=(et == 0),
                    stop=(et == NE - 1),
                )
            if on_scalar:
                nc.scalar.copy(out=dst[:, d0 : d0 + 512], in_=acc[0:wm, 0:512])
            else:
                nc.vector.tensor_copy(
                    out=dst[:, d0 : d0 + 512], in_=acc[0:wm, 0:512]
                )

        def shift(dst, src, d0):
            # scalar's DMA queue is otherwise idle until the exps begin,
            # so the shifts never wait behind bulk input pieces.
            nc.scalar.dma_start(
                out=dst[:, d0 : d0 + 512], in_=src[64:128, d0 : d0 + 512]
            )

        def v_transpose(k):
            tp = psA.tile([128, 1024], BF16, tag="ps")
            if k < 16:  # own half: vt carries the ones row
                nc.tensor.transpose(
                    out=tp[0:128, 0:65],
                    in_=vt[:, k * 128 : (k + 1) * 128],
                    identity=identB[0:65, 0:65],
                )
                nc.vector.tensor_copy(out=vn[:, k, :], in_=tp[0:128, 0:65])
            else:  # other half: V^T shifted into vto (base partition 0)
                j = k - 16
                nc.tensor.transpose(
                    out=tp[0:128, 0:64],
                    in_=vto[:, j * 128 : (j + 1) * 128],
                    identity=identB[0:64, 0:64],
                )
                nc.vector.memset(vn[:, k, D : D + 1], 1.0)
                nc.vector.tensor_copy(out=vn[:, k, 0:D], in_=tp[0:128, 0:64])

        # --- prologue: pass A halves 0-1 (K^T + Q^T own, q-cols 0:1024) ---
        for hh in range(2):
            proj_half(0, 128, qk, hh * 512, hh * 512)
            shift(qts, qk, hh * 512)

        # side-slot schedule for pass 0: iter k -> (kind, half-index).
        # Slots follow the input-DMA arrival order: own cols 0:1024 first
        # (C0/C1), then own 1024:2048 (~iter 5-6: A2/C2/A3/C3), then the
        # other half (~iter 12-13: B lumps). "F" = junk matmuls that keep
        # the PE streaming through the group-2 DMA wait (a >2.5us idle
        # makes the HAM duty controller halve the PE clock).
        SIDE = {
            1: ("C", 0), 3: ("C", 1),
            6: ("A", 2), 7: ("C", 2), 10: ("A", 3), 11: ("C", 3),
            14: ("B", 0), 16: ("B", 1), 18: ("B", 2), 20: ("B", 3),
        }

        def side_work(k):
            s = SIDE.get(k)
            if s is not None:
                kind, hh = s
                if kind == "A":
                    proj_half(0, 128, qk, hh * 512, hh * 512, on_scalar=True)
                    shift(qts, qk, hh * 512)
                elif kind == "C":
                    proj_half(256, 64, vt[0:64, :], hh * 512, hh * 512)
                else:
                    proj_half(128, 128, kv, HALF + hh * 512, hh * 512,
                              on_scalar=True)
                    shift(vto, kv, hh * 512)
            if k == 2:
                v_transpose(0)
                v_transpose(1)
            elif k >= 3:
                v_transpose(k - 1)
                if k == NKT - 1:
                    v_transpose(NKT - 1)

        out_engs = [nc.sync, nc.gpsimd]

        # --- two q-passes of 1024 columns each ---
        for ps in range(2):
            att_ps = psB.tile([128, 1024], FP32)
            p_tiles = {}

            for k in range(NKT):
                if k < 16:
                    klhs = qk[0:64, k * 128 : (k + 1) * 128]
                else:
                    klhs = kv[0:64, (k - 16) * 128 : (k - 15) * 128]

                sc = psA.tile([128, 1024], FP32, tag="ps")
                if ps == 0 and k in (4, 5):
                    # DMA-wait fillers: keep the PE streaming while the
                    # group-2 input pieces land (overwritten by the
                    # start=True scores matmuls below).
                    for _ in range(4):
                        nc.tensor.matmul(
                            out=sc[:, 0:512],
                            lhsT=junk[:, 0:128],
                            rhs=junk[:, :],
                            start=True,
                            stop=True,
                        )
                for c in range(2):
                    q0 = ps * 1024 + c * 512
                    nc.tensor.matmul(
                        out=sc[:, c * 512 : (c + 1) * 512],
                        lhsT=klhs,
                        rhs=qts[:, q0 : q0 + 512],
                        start=True,
                        stop=True,
                    )
                eng = _exp_eng(k)
                if eng == "S":
                    p = pps.tile([128, 1024], BF16)
                    nc.scalar.activation(
                        out=p[:, :], in_=sc[:, :], func=Exp, scale=SCALE
                    )
                else:
                    p = ppv.tile([128, 1024], BF16)
                    nc.vector.tensor_scalar(
                        p[:, :].bitcast(I16), sc[:, :], A16, B16, Mult, Add
                    )
                p_tiles[k] = p

                if ps == 0:
                    side_work(k)
                if k >= 2:
                    _attn(nc, att_ps, vn, p_tiles, k - 2)

            _attn(nc, att_ps, vn, p_tiles, NKT - 2)
            _attn(nc, att_ps, vn, p_tiles, NKT - 1)

            # ship this pass's attn^T + denominators (host normalizes)
            for c in range(2):
                cols = slice(ps * 1024 + c * 512, ps * 1024 + (c + 1) * 512)
                pcols = slice(c * 512, (c + 1) * 512)
                nc.vector.tensor_copy(out=att_sb[:, cols], in_=att_ps[0:65, pcols])
                out_engs[c].dma_start(out=out_d[:, cols], in_=att_sb[:, cols])

    nc.compile()
    _CACHE["nc"] = nc
    return nc


def _attn(nc, att_ps, vn, p_tiles, k):
    p = p_tiles.pop(k)
    for c in range(2):
        nc.tensor.matmul(
            out=att_ps[0:65, c * 512 : (c + 1) * 512],
            lhsT=vn[:, k, :],
            rhs=p[:, c * 512 : (c + 1) * 512],
            start=(k == 0),
            stop=(k == NKT - 1),
            skip_group_check=True,
        )


def _make_in_maps(x, Wq, Wk, Wv):
    import ml_dtypes

    bf16 = ml_dtypes.bfloat16
    xT = np.ascontiguousarray(x.transpose(0, 2, 1)).astype(bf16)  # [B, E, S]
    wt = np.concatenate(
        [Wk.T, Wq.T, Wk.T, Wv.T, Wv.T], axis=1
    ).astype(bf16)  # [E, 320]
    in_maps = []
    for c in range(N_CORES):
        b, h = divmod(c, 2)
        in_maps.append(
            {
                "xt_q": np.ascontiguousarray(xT[b, :, h * HALF : (h + 1) * HALF]),
                "xt_o": np.ascontiguousarray(
                    xT[b, :, (1 - h) * HALF : (2 - h) * HALF]
                ),
                "wt": wt,
            }
        )
    return in_maps


def _run(x, Wq, Wk, Wv, trace=False):
    from concourse.bass_utils import run_bass_kernel_spmd

    nc = _build()
    in_maps = _make_in_maps(x, Wq, Wk, Wv)
    res = run_bass_kernel_spmd(
        nc, in_maps, core_ids=list(range(N_CORES)), trace=trace
    )
    out = np.empty((B, S, D), dtype=np.float32)
    for c in range(N_CORES):
        b, h = divmod(c, 2)
        att = res.results[c]["out"]  # [65, HALF]: attn^T rows + denom row
        out[b, h * HALF : (h + 1) * HALF, :] = (att[0:D] / att[D : D + 1]).T
    return out, res


def kernel(x, Wq, Wk, Wv):
    out, _ = _run(
        np.asarray(x, dtype=np.float32),
        np.asarray(Wq, dtype=np.float32),
        np.asarray(Wk, dtype=np.float32),
        np.asarray(Wv, dtype=np.float32),
    )
    return out



# revision 46
# speedup vs baseline: 1.0022x; 1.0022x over previous
"""Single-head attention (B=4, S=4096, E=1024, D=64) on 8 TRN2 NeuronCores.

Sharding: data-parallel over (batch, query-half): core c handles batch
b = c // 2 and query rows [h*2048, (h+1)*2048) with h = c % 2. Each core
loads ONLY its own half of x^T (4MB instead of 8MB) and computes K/V for
its own 2048 rows; the two cores of a batch EXCHANGE their packed
[K^T; V^T] blocks through an HBM AllGather over replica pairs. The
partner's block is pulled from the gathered buffer with an indirect
(index-vector) DMA whose row indices come from a per-core host input —
that lets a single SPMD program address "the other core's slot" without
branching.

The AllGather costs ~20us end-to-end, so the k-loop is restructured to
hide it: OWN k-tiles (0..15) are processed for BOTH q-passes first
(~35us of exchange-independent work, pass 1 lagging pass 0 by 4 iters
while its Q columns project), then the OTHER k-tiles (16..31) for both
passes once the partner's block has landed. Both attention accumulators
live in PSUM simultaneously: psB holds att0+att1 (4 banks), psA is cut
to 2 score slots (4 banks) = all 8 banks.

Per-core dataflow (TensorE bf16; fp32 accumulation in PSUM):
  kvown [128, 2048] = [K^T_own; V^T_own]   (pass AB, lhsT = [WkT | WvT])
  qts   [64, 2048]  = Q^T_own              (pass Q, lhsT = WqT, base 0)
  kvoth [128, 2048] = partner's [K^T; V^T] (AllGather + indirect gather)
  scores^T[k, q] = K^T.T @ Q^T -> exp -> P bf16
  attn^T[65, q] += V_aug.T @ P   (row 64 accumulates softmax denominators)
V rows (64:128 of kvown/kvoth) are PE-transposed per k-tile into natural
orientation vn[128, k, 65]; vn's ones column is memset once up front.

The per-iteration exp pair (one per pass) is split one-ScalarE (exact ACT
exp) + one-VectorE (one-instruction Schraudolph bit-trick: i16 =
rne(x*A + B) bitcast bf16 ~= exp(SCALE*x); ~1.8% mantissa-chord sawtooth
whose mean bias cancels in the softmax num/denom ratio; on ~32/64 of the
weights it adds ~0.8% output rel err). Separate P pools per engine avoid
buffer-reuse serialization; AB projection copies run on ScalarE to keep
VectorE's in-order queue (exps + transpose copies) from backing up.

The HAM duty controller halves the PE clock after ~2.5us of PE idleness
(5-13us at half clock to re-grant), so junk-fed 512-col warm-up matmuls
stream from the instant the PE preamble ends until the first input DMAs
land, and junk fillers bridge the group-2 DMA wait.
"""

import numpy as np

B, S, E, D = 4, 4096, 1024, 64
HALF = S // 2
N_CORES = 8
SCALE = 1.0 / np.sqrt(D)

NE = E // 128  # 8 e-tiles
NKT = S // 128  # 32 k-tiles
N_WARM = 20  # 512-col PE warm-up matmuls covering the preamble + DMA wait
LAG1 = 4  # pass-1 iteration lag behind pass 0 in the own-tile phase

# Schraudolph exp-approx constants (bf16 bit pattern via int16):
#   i16 = round(x * A16 + B16); bitcast bf16 ~= exp(SCALE * x)
LOG2E = 1.4426950408889634
A16 = SCALE * 128.0 * LOG2E
B16 = 127.0 * 128.0 - 7.3


def _exp_eng(ps, k):
    """One scalar + one vector exp per (k, k) iteration pair."""
    if ps == 0:
        return "V" if k % 3 == 2 else "S"
    return "S" if k % 3 == 2 else "V"

_CACHE = {}


def _build():
    if "nc" in _CACHE:
        return _CACHE["nc"]

    from contextlib import ExitStack

    import concourse.bacc as bacc
    import concourse.bass as bass
    import concourse.tile as tile
    from concourse import mybir
    from concourse.masks import make_identity

    FP32 = mybir.dt.float32
    BF16 = mybir.dt.bfloat16
    I16 = mybir.dt.int16
    I32 = mybir.dt.int32
    Exp = mybir.ActivationFunctionType.Exp
    Mult = mybir.AluOpType.mult
    Add = mybir.AluOpType.add

    nc = bacc.Bacc(
        "TRN2", target_bir_lowering=False, debug=False, num_devices=N_CORES
    )

    xt_q_d = nc.dram_tensor("xt_q", [E, HALF], BF16, kind="ExternalInput").ap()
    wt_d = nc.dram_tensor("wt", [E, 192], BF16, kind="ExternalInput").ap()
    vidx_d = nc.dram_tensor("vidx", [128, 1], I32, kind="ExternalInput").ap()
    out_d = nc.dram_tensor("out", [D + 1, HALF], FP32, kind="ExternalOutput").ap()

    with tile.TileContext(nc) as tc, ExitStack() as ctx:
        const = ctx.enter_context(tc.tile_pool(name="const", bufs=1))
        big = ctx.enter_context(tc.tile_pool(name="big", bufs=1))
        pps = ctx.enter_context(tc.tile_pool(name="pps", bufs=4))
        ppv = ctx.enter_context(tc.tile_pool(name="ppv", bufs=4))
        psA = ctx.enter_context(tc.tile_pool(name="psA", bufs=2, space="PSUM"))
        psB = ctx.enter_context(tc.tile_pool(name="psB", bufs=1, space="PSUM"))
        dram = ctx.enter_context(tc.tile_pool(name="dram", bufs=2, space="DRAM"))

        identB = const.tile([128, 128], BF16)
        junk = const.tile([128, 512], BF16)
        vidx = const.tile([128, 1], I32)
        nc.vector.memset(junk[:, :], 0.0)
        make_identity(nc, identB)

        xt = big.tile([128, NE, HALF], BF16)  # x^T, own q-half only
        wt = big.tile([128, NE, 192], BF16)  # [WkT | WvT | WqT]
        kvown = big.tile([128, HALF], BF16)  # rows 0:64 K^T, 64:128 V^T own
        kvoth = big.tile([128, HALF], BF16)  # partner's [K^T; V^T]
        qts = big.tile([64, HALF], BF16)  # Q^T own at base partition 0
        vn = big.tile([128, NKT, D + 1], BF16)  # V natural + ones column
        att_sb = big.tile([65, HALF], FP32)  # attn^T + denominator row

        ccin = dram.tile([128, HALF], BF16)  # my [K^T; V^T] bounce
        ccout = dram.tile([2 * 128, HALF], BF16)  # gathered pair

        # --- PE warm-up (HAM: never let the PE idle >2.5us) ---
        warm = psA.tile([128, 1024], FP32, tag="ps")
        for _ in range(N_WARM):
            nc.tensor.matmul(
                out=warm[0:128, 0:512],
                lhsT=junk[:, 0:128],
                rhs=junk[:, :],
                start=True,
                stop=True,
            )

        # --- input DMAs in deadline order: wt + own cols 0:1024 (gates
        # prologue AB0/Q0/Q1), then own 1024:2048 (AB/Q lumps). No "other
        # half" group: the partner's K/V arrive over the AllGather.
        nc.sync.dma_start(out=wt[:, :, :], in_=wt_d.rearrange("(t p) d -> p t d", p=128))
        nc.sync.dma_start(out=vidx[:, :], in_=vidx_d[:, :])
        first_engs = [nc.gpsimd, nc.sync, nc.gpsimd, nc.scalar,
                      nc.gpsimd, nc.sync, nc.gpsimd, nc.scalar]
        for et in range(NE):
            first_engs[et].dma_start(
                out=xt[:, et, 0:1024],
                in_=xt_q_d[et * 128 : (et + 1) * 128, 0:1024],
            )
        for et in range(NE):
            eng = [nc.gpsimd, nc.sync][et % 2]
            eng.dma_start(
                out=xt[:, et, 1024:2048],
                in_=xt_q_d[et * 128 : (et + 1) * 128, 1024:2048],
            )

        nc.vector.memset(vn[:, :, D : D + 1], 1.0)

        # one packed projection half-chunk of 512 cols; AB copies go to
        # ScalarE (its `copy` shares the exp act-table set) so VectorE's
        # in-order queue never delays the exchange bounce.
        def proj_half(w0, wm, dst, src_x0, d0, on_scalar=False):
            acc = psA.tile([128, 1024], FP32, tag="ps")
            for et in range(NE):
                nc.tensor.matmul(
                    out=acc[0:wm, 0:512],
                    lhsT=wt[:, et, w0 : w0 + wm],
                    rhs=xt[:, et, src_x0 : src_x0 + 512],
                    start=(et == 0),
                    stop=(et == NE - 1),
                )
            if on_scalar:
                nc.scalar.copy(out=dst[:, d0 : d0 + 512], in_=acc[0:wm, 0:512])
            else:
                nc.vector.tensor_copy(
                    out=dst[:, d0 : d0 + 512], in_=acc[0:wm, 0:512]
                )

        def v_transpose(k):
            # V^T rows live at base partition 64 of kvown/kvoth; transpose
            # against the base-64 diagonal block of identB.
            tp = psA.tile([128, 1024], BF16, tag="ps")
            src = kvown if k < 16 else kvoth
            j = k if k < 16 else k - 16
            nc.tensor.transpose(
                out=tp[0:128, 0:64],
                in_=src[64:128, j * 128 : (j + 1) * 128],
                identity=identB[64:128, 64:128],
            )
            nc.vector.tensor_copy(out=vn[:, k, 0:D], in_=tp[0:128, 0:64])

        # --- prologue: AB half 0 (K/V own cols 0:512) + Q halves 0-1 ---
        proj_half(0, 128, kvown, 0, 0, on_scalar=True)
        for hh in range(2):
            proj_half(128, 64, qts, hh * 512, hh * 512)

        # phase-1 side slots: AB lumps early (they gate the exchange),
        # Q2/Q3 before pass-1 scores start at iter LAG1, exchange at 6.
        SIDE = {
            0: ("AB", 1), 1: ("Q", 2), 2: ("Q", 3), 3: ("AB", 2),
            5: ("AB", 3), 6: ("X", 0),
        }

        def side_work(i):
            s = SIDE.get(i)
            if s is not None:
                kind, hh = s
                if kind == "AB":
                    proj_half(0, 128, kvown, hh * 512, hh * 512, on_scalar=True)
                elif kind == "Q":
                    proj_half(128, 64, qts, hh * 512, hh * 512)
                else:  # X: pair exchange of [K^T; V^T] through HBM
                    nc.sync.dma_start(out=ccin[:, :], in_=kvown[:, :])
                    nc.gpsimd.collective_compute(
                        "AllGather",
                        mybir.AluOpType.bypass,
                        replica_groups=[[0, 1], [2, 3], [4, 5], [6, 7]],
                        ins=[ccin[:, :]],
                        outs=[ccout[:, :]],
                    )
                    nc.gpsimd.indirect_dma_start(
                        out=kvoth[:, :],
                        out_offset=None,
                        in_=ccout[:, :],
                        in_offset=bass.IndirectOffsetOnAxis(
                            ap=vidx[:, 0:1], axis=0
                        ),
                    )
            if i == 2:
                v_transpose(0)
                v_transpose(1)
            elif i >= 3 and i - 1 < 16:
                v_transpose(i - 1)

        att0 = psB.tile([128, 1024], FP32)
        att1 = psB.tile([128, 1024], FP32)
        p_tiles = {0: {}, 1: {}}

        def do_k(ps, k, n_fill=0):
            klhs = (
                kvown[0:64, k * 128 : (k + 1) * 128]
                if k < 16
                else kvoth[0:64, (k - 16) * 128 : (k - 15) * 128]
            )
            sc = psA.tile([128, 1024], FP32, tag="ps")
            for _ in range(n_fill):
                nc.tensor.matmul(
                    out=sc[:, 0:512],
                    lhsT=junk[:, 0:128],
                    rhs=junk[:, :],
                    start=True,
                    stop=True,
                )
            for c in range(2):
                q0 = ps * 1024 + c * 512
                nc.tensor.matmul(
                    out=sc[:, c * 512 : (c + 1) * 512],
                    lhsT=klhs,
                    rhs=qts[:, q0 : q0 + 512],
                    start=True,
                    stop=True,
                )
            if _exp_eng(ps, k) == "S":
                p = pps.tile([128, 1024], BF16)
                nc.scalar.activation(
                    out=p[:, :], in_=sc[:, :], func=Exp, scale=SCALE
                )
            else:
                p = ppv.tile([128, 1024], BF16)
                nc.vector.tensor_scalar(
                    p[:, :].bitcast(I16), sc[:, :], A16, B16, Mult, Add
                )
            p_tiles[ps][k] = p

        def attn_do(ps, k):
            _attn(nc, att0 if ps == 0 else att1, vn, p_tiles[ps], k)

        # --- phase 1: own k-tiles (0..15), both passes, ps1 lags LAG1 ---
        for i in range(16 + LAG1):
            side_work(i)
            if i < 16:
                do_k(0, i, n_fill=4 if i in (2, 3) else 0)
            if i >= LAG1:
                do_k(1, i - LAG1)
            if 2 <= i and i - 2 < 16:
                attn_do(0, i - 2)
            if i - LAG1 - 2 >= 0:
                attn_do(1, i - LAG1 - 2)

        # --- phase 2: other k-tiles (16..31), both passes ---
        for k in range(16, NKT):
            if k == 16:
                v_transpose(16)
                v_transpose(17)
            elif k + 1 < NKT:
                v_transpose(k + 1)
            do_k(0, k)
            do_k(1, k)
            if k - 2 >= 16:
                attn_do(0, k - 2)
            attn_do(1, k - 2)

        for ps in range(2):
            attn_do(ps, NKT - 2)
            attn_do(ps, NKT - 1)

        # ship both passes' attn^T + denominators (host normalizes)
        out_engs = [nc.sync, nc.gpsimd, nc.sync, nc.gpsimd]
        for ps in range(2):
            att_ps = att0 if ps == 0 else att1
            for c in range(2):
                cols = slice(ps * 1024 + c * 512, ps * 1024 + (c + 1) * 512)
                pcols = slice(c * 512, (c + 1) * 512)
                nc.vector.tensor_copy(out=att_sb[:, cols], in_=att_ps[0:65, pcols])
                out_engs[ps * 2 + c].dma_start(out=out_d[:, cols], in_=att_sb[:, cols])

    nc.compile()
    _CACHE["nc"] = nc
    return nc


def _attn(nc, att_ps, vn, p_tiles, k):
    p = p_tiles.pop(k)
    for c in range(2):
        nc.tensor.matmul(
            out=att_ps[0:65, c * 512 : (c + 1) * 512],
            lhsT=vn[:, k, :],
            rhs=p[:, c * 512 : (c + 1) * 512],
            start=(k == 0),
            stop=(k == NKT - 1),
            skip_group_check=True,
        )


def _make_in_maps(x, Wq, Wk, Wv):
    import ml_dtypes

    bf16 = ml_dtypes.bfloat16
    xT = np.ascontiguousarray(x.transpose(0, 2, 1)).astype(bf16)  # [B, E, S]
    wt = np.concatenate([Wk.T, Wv.T, Wq.T], axis=1).astype(bf16)  # [E, 192]
    in_maps = []
    for c in range(N_CORES):
        b, h = divmod(c, 2)
        vidx = (128 * (1 - h) + np.arange(128, dtype=np.int32)).reshape(128, 1)
        in_maps.append(
            {
                "xt_q": np.ascontiguousarray(xT[b, :, h * HALF : (h + 1) * HALF]),
                "wt": wt,
                "vidx": vidx,
            }
        )
    return in_maps


def _run(x, Wq, Wk, Wv, trace=False):
    from concourse.bass_utils import run_bass_kernel_spmd

    nc = _build()
    in_maps = _make_in_maps(x, Wq, Wk, Wv)
    res = run_bass_kernel_spmd(
        nc, in_maps, core_ids=list(range(N_CORES)), trace=trace
    )
    out = np.empty((B, S, D), dtype=np.float32)
    for c in range(N_CORES):
        b, h = divmod(c, 2)
        att = res.results[c]["out"]  # [65, HALF]: attn^T rows + denom row
        out[b, h * HALF : (h + 1) * HALF, :] = (att[0:D] / att[D : D + 1]).T
    return out, res


def kernel(x, Wq, Wk, Wv):
    out, _ = _run(
        np.asarray(x, dtype=np.float32),
        np.asarray(Wq, dtype=np.float32),
        np.asarray(Wk, dtype=np.float32),
        np.asarray(Wv, dtype=np.float32),
    )
    return out


# revision 48
# speedup vs baseline: 1.1983x; 1.1958x over previous
"""Single-head attention (B=4, S=4096, E=1024, D=64) on 8 TRN2 NeuronCores.

Sharding: data-parallel over (batch, query-half): core c handles batch
b = c // 2 and query rows [h*2048, (h+1)*2048) with h = c % 2. Each core
computes Q for its own 2048 rows and K/V for the full 4096 rows of its batch
(inputs are shipped host-pretransposed per half, so no duplicated DMA).

Per-core dataflow (TensorE matmuls in bf16 — fp32/fp32r matmuls run the PE
at half clock; fp32 accumulation in PSUM). Projections pack TWO weight
matrices into one 128-wide stationary operand:
  qk [128, 2048] = [K^T_own; Q^T_own]     (pass A, lhsT = [WkT | WqT])
  kv [128, 2048] = [K^T_oth; V^T_oth]     (pass B, lhsT = [WkT | WvT])
  vt [65, 2048]  = V^T_own + ones row      (pass C, lhsT = WvT)
Q^T / V^T_oth are then shifted to base partition 0 by SBUF-to-SBUF DMAs
(matmul operands must share a base partition).
  scores^T[k, q] = K^T.T @ Q^T -> exp -> P bf16
  attn^T[65, q] += V_aug.T @ P   (row 64 accumulates softmax denominators)
  output = attn^T with denominators; host transposes + normalizes.

The exp is split across two engines so ScalarE (1 elem/cycle/lane at
1.2 GHz, ~1.15us per [128,1024] tile) stops pacing the pipeline: 2/3 of
k-tiles get the exact ACT exp on ScalarE; every third tile is computed on
VectorE with a one-instruction Schraudolph bit-trick: i16 = rne(x*A + B)
reinterpreted as bf16 approximates exp(SCALE*x) (piecewise-linear mantissa
chord, ~1.8% log-error sawtooth whose mean bias cancels in the softmax
numerator/denominator ratio; applied to 20/64 of the weights it adds
~0.6% output rel err). The two engines use SEPARATE P-tile pools — a
shared pool serializes them via buffer-reuse ordering.

The attention runs as TWO passes over q (1024 columns each): the attn
accumulator then fits 2 PSUM banks, freeing a third scores slot (PSUM slot
contention paced the single-pass version), and pass 0's output ships
mid-kernel.

The HAM duty controller halves the PE clock after ~2.5us of PE idleness
and takes 5-13us at half clock to re-grant full duty, so the kernel keeps
the PE streaming: junk-fed 512-col warm-up matmuls run from the instant
the PE preamble ends until the first input DMAs land (gated only on a
VectorE memset, not on make_identity's gpsimd iota), and junk fillers
bridge the group-2 DMA wait at pass-0 iters 4-5. Input DMA is issued in
deadline order (wt + own q-cols 0:1024, then own 1024:2048, then the
other half) across the sync/scalar/gpsimd queues; the pass-0 side-slot
schedule (projection lumps A2/C2/A3/C3, then B0-B3) tracks the measured
arrival of those groups.
"""

import numpy as np

B, S, E, D = 4, 4096, 1024, 64
HALF = S // 2
N_CORES = 8
SCALE = 1.0 / np.sqrt(D)

NE = E // 128  # 8 e-tiles
NKT = S // 128  # 32 k-tiles
N_WARM = 20  # 512-col PE warm-up matmuls covering the preamble + DMA wait

# Schraudolph exp-approx constants (bf16 bit pattern via int16):
#   i16 = round(x * A16 + B16); bitcast bf16 ~= exp(SCALE * x)
LOG2E = 1.4426950408889634
A16 = SCALE * 128.0 * LOG2E
B16 = 127.0 * 128.0 - 7.3


def _exp_eng(k):
    """Which engine computes exp for k-tile `k` (0..NKT-1) of a pass.

    'S' = ScalarE exact ACT exp; 'V' = VectorE Schraudolph. (GpSimd
    cannot read PSUM, so it can't help here.) 22 exact + 10 approx per
    pass: the approx noise (~1.8% per weight) lands on 20/64 of the
    softmax weights -> ~0.6% output rel err.
    """
    return "V" if k % 3 == 2 else "S"

_CACHE = {}


def _build():
    if "nc" in _CACHE:
        return _CACHE["nc"]

    from contextlib import ExitStack

    import concourse.bacc as bacc
    import concourse.tile as tile
    from concourse import mybir
    from concourse.masks import make_identity

    FP32 = mybir.dt.float32
    BF16 = mybir.dt.bfloat16
    I16 = mybir.dt.int16
    Exp = mybir.ActivationFunctionType.Exp
    Mult = mybir.AluOpType.mult
    Add = mybir.AluOpType.add

    nc = bacc.Bacc(
        "TRN2", target_bir_lowering=False, debug=False, num_devices=N_CORES
    )

    xt_q_d = nc.dram_tensor("xt_q", [E, HALF], BF16, kind="ExternalInput").ap()
    xt_o_d = nc.dram_tensor("xt_o", [E, HALF], BF16, kind="ExternalInput").ap()
    wt_d = nc.dram_tensor("wt", [E, 320], BF16, kind="ExternalInput").ap()
    out_d = nc.dram_tensor("out", [D + 1, HALF], FP32, kind="ExternalOutput").ap()

    with tile.TileContext(nc) as tc, ExitStack() as ctx:
        const = ctx.enter_context(tc.tile_pool(name="const", bufs=1))
        big = ctx.enter_context(tc.tile_pool(name="big", bufs=1))
        # separate P pools per exp engine: a shared pool serializes the
        # scalar and vector exps against each other via slot-reuse ordering
        pps = ctx.enter_context(tc.tile_pool(name="pps", bufs=4))
        ppv = ctx.enter_context(tc.tile_pool(name="ppv", bufs=3))
        psA = ctx.enter_context(tc.tile_pool(name="psA", bufs=3, space="PSUM"))
        psB = ctx.enter_context(tc.tile_pool(name="psB", bufs=1, space="PSUM"))

        identB = const.tile([128, 128], BF16)
        junk = const.tile([128, 512], BF16)
        # memset on vector: ready right at the end of vector's preamble, so
        # the PE warm-up can start ~2.5us before make_identity's gpsimd
        # iota would allow.
        nc.vector.memset(junk[:, :], 0.0)
        make_identity(nc, identB)

        xt = big.tile([128, NE, S], BF16)  # x^T; cols [0, HALF) = own q-rows
        wt = big.tile([128, NE, 320], BF16)  # [WkT|WqT | WkT|WvT | WvT]
        qk = big.tile([128, HALF], BF16)  # rows 0-63 K^T own, 64-127 Q^T own
        kv = big.tile([128, HALF], BF16)  # rows 0-63 K^T oth, 64-127 V^T oth
        qts = big.tile([64, HALF], BF16)  # Q^T shifted to base partition 0
        vto = big.tile([64, HALF], BF16)  # V^T other shifted to base part. 0
        vt = big.tile([65, HALF], BF16)  # V^T own; row 64 = ones
        vn = big.tile([128, NKT, D + 1], BF16)  # V natural + ones column
        att_sb = big.tile([65, HALF], FP32)  # attn^T + denominator row

        # --- PE warm-up: the HAM duty controller halves the PE clock after
        # ~2.5us of idleness and takes 5-13us at half clock to restore full
        # duty, so keep the PE streaming junk matmuls from the instant its
        # preamble ends until the first input DMAs land.
        warm = psA.tile([128, 1024], FP32, tag="ps")
        for _ in range(N_WARM):
            nc.tensor.matmul(
                out=warm[0:128, 0:512],
                lhsT=junk[:, 0:128],
                rhs=junk[:, :],
                start=True,
                stop=True,
            )

        # --- input DMAs. One dma_start ~= one DMA queue, so split per
        # e-tile. Priority: wt, own cols 0:1024 (gates pass A / first exp),
        # then own cols 1024:2048 (A2/A3 lumps, ~iter 5+), then the other
        # half (B lumps, ~iter 13+). sync stays free-ish for the shifts.
        nc.sync.dma_start(out=wt[:, :, :], in_=wt_d.rearrange("(t p) d -> p t d", p=128))
        # cols 0:1024 (gates the first exp): spread across all three
        # DMA-capable engines; gpsimd's SWDGE has multiple queues.
        first_engs = [nc.gpsimd, nc.sync, nc.gpsimd, nc.scalar,
                      nc.gpsimd, nc.sync, nc.gpsimd, nc.scalar]
        for et in range(NE):
            first_engs[et].dma_start(
                out=xt[:, et, 0:1024],
                in_=xt_q_d[et * 128 : (et + 1) * 128, 0:1024],
            )
        for et in range(NE):
            nc.gpsimd.dma_start(
                out=xt[:, et, 1024:2048],
                in_=xt_q_d[et * 128 : (et + 1) * 128, 1024:2048],
            )
        for et in range(NE):
            eng = [nc.gpsimd, nc.sync][et % 2]
            eng.dma_start(
                out=xt[:, et, HALF:S],
                in_=xt_o_d[et * 128 : (et + 1) * 128, :],
            )

        nc.vector.memset(vt[64:65, :], 1.0)

        # one packed projection half-chunk of 512 cols
        def proj_half(w0, wm, dst, src_x0, d0):
            acc = psA.tile([128, 1024], FP32, tag="ps")
            for et in range(NE):
                nc.tensor.matmul(
                    out=acc[0:wm, 0:512],
                    lhsT=wt[:, et, w0 : w0 + wm],
                    rhs=xt[:, et, src_x0 : src_x0 + 512],
                    start=(et == 0),
                    stop=(et == NE - 1),
                )
            nc.vector.tensor_copy(out=dst[:, d0 : d0 + 512], in_=acc[0:wm, 0:512])

        def shift(dst, src, d0):
            # scalar's DMA queue is otherwise idle until the exps begin,
            # so the shifts never wait behind bulk input pieces.
            nc.scalar.dma_start(
                out=dst[:, d0 : d0 + 512], in_=src[64:128, d0 : d0 + 512]
            )

        def v_transpose(k):
            tp = psA.tile([128, 1024], BF16, tag="ps")
            if k < 16:  # own half: vt carries the ones row
                nc.tensor.transpose(
                    out=tp[0:128, 0:65],
                    in_=vt[:, k * 128 : (k + 1) * 128],
                    identity=identB[0:65, 0:65],
                )
                nc.vector.tensor_copy(out=vn[:, k, :], in_=tp[0:128, 0:65])
            else:  # other half: V^T shifted into vto (base partition 0)
                j = k - 16
                nc.tensor.transpose(
                    out=tp[0:128, 0:64],
                    in_=vto[:, j * 128 : (j + 1) * 128],
                    identity=identB[0:64, 0:64],
                )
                nc.vector.memset(vn[:, k, D : D + 1], 1.0)
                nc.vector.tensor_copy(out=vn[:, k, 0:D], in_=tp[0:128, 0:64])

        # --- prologue: pass A halves 0-1 (K^T + Q^T own, q-cols 0:1024) ---
        for hh in range(2):
            proj_half(0, 128, qk, hh * 512, hh * 512)
            shift(qts, qk, hh * 512)

        # side-slot schedule for pass 0: iter k -> (kind, half-index).
        # Slots follow the input-DMA arrival order: own cols 0:1024 first
        # (C0/C1), then own 1024:2048 (~iter 5-6: A2/C2/A3/C3), then the
        # other half (~iter 12-13: B lumps).
        SIDE = {
            1: ("C", 0), 3: ("C", 1),
            6: ("A", 2), 7: ("C", 2), 10: ("A", 3), 11: ("C", 3),
            14: ("B", 0), 16: ("B", 1), 18: ("B", 2), 20: ("B", 3),
        }

        def side_work(k):
            s = SIDE.get(k)
            if s is not None:
                kind, hh = s
                if kind == "A":
                    proj_half(0, 128, qk, hh * 512, hh * 512)
                    shift(qts, qk, hh * 512)
                elif kind == "C":
                    proj_half(256, 64, vt[0:64, :], hh * 512, hh * 512)
                else:
                    proj_half(128, 128, kv, HALF + hh * 512, hh * 512)
                    shift(vto, kv, hh * 512)
            if k == 2:
                v_transpose(0)
                v_transpose(1)
            elif k >= 3:
                v_transpose(k - 1)
                if k == NKT - 1:
                    v_transpose(NKT - 1)

        out_engs = [nc.sync, nc.gpsimd]

        # --- two q-passes of 1024 columns each ---
        for ps in range(2):
            att_ps = psB.tile([128, 1024], FP32)
            p_tiles = {}

            for k in range(NKT):
                if k < 16:
                    klhs = qk[0:64, k * 128 : (k + 1) * 128]
                else:
                    klhs = kv[0:64, (k - 16) * 128 : (k - 15) * 128]

                sc = psA.tile([128, 1024], FP32, tag="ps")
                if ps == 0 and k in (4, 5):
                    # DMA-wait fillers: keep the PE streaming while the
                    # group-2 input pieces land (overwritten by the
                    # start=True scores matmuls below).
                    for _ in range(4):
                        nc.tensor.matmul(
                            out=sc[:, 0:512],
                            lhsT=junk[:, 0:128],
                            rhs=junk[:, :],
                            start=True,
                            stop=True,
                        )
                for c in range(2):
                    q0 = ps * 1024 + c * 512
                    nc.tensor.matmul(
                        out=sc[:, c * 512 : (c + 1) * 512],
                        lhsT=klhs,
                        rhs=qts[:, q0 : q0 + 512],
                        start=True,
                        stop=True,
                    )
                eng = _exp_eng(k)
                if eng == "S":
                    p = pps.tile([128, 1024], BF16)
                    nc.scalar.activation(
                        out=p[:, :], in_=sc[:, :], func=Exp, scale=SCALE
                    )
                else:
                    p = ppv.tile([128, 1024], BF16)
                    nc.vector.tensor_scalar(
                        p[:, :].bitcast(I16), sc[:, :], A16, B16, Mult, Add
                    )
                p_tiles[k] = p

                if ps == 0:
                    side_work(k)
                # attn lags THREE iters behind scores: at lag 2 the exp
                # chain (ScalarE ~1.15us, 2 of every 3 tiles) misses the
                # deadline by ~230ns every iteration and paces the whole
                # pipeline at 1085ns/iter instead of the PE-bound 853ns.
                if k >= 3:
                    _attn(nc, att_ps, vn, p_tiles, k - 3)

            for kt in (NKT - 3, NKT - 2, NKT - 1):
                _attn(nc, att_ps, vn, p_tiles, kt)

            # ship this pass's attn^T + denominators (host normalizes)
            for c in range(2):
                cols = slice(ps * 1024 + c * 512, ps * 1024 + (c + 1) * 512)
                pcols = slice(c * 512, (c + 1) * 512)
                nc.vector.tensor_copy(out=att_sb[:, cols], in_=att_ps[0:65, pcols])
                out_engs[c].dma_start(out=out_d[:, cols], in_=att_sb[:, cols])

    nc.compile()
    _CACHE["nc"] = nc
    return nc


def _attn(nc, att_ps, vn, p_tiles, k):
    p = p_tiles.pop(k)
    for c in range(2):
        nc.tensor.matmul(
            out=att_ps[0:65, c * 512 : (c + 1) * 512],
            lhsT=vn[:, k, :],
            rhs=p[:, c * 512 : (c + 1) * 512],
            start=(k == 0),
            stop=(k == NKT - 1),
            skip_group_check=True,
        )


def _make_in_maps(x, Wq, Wk, Wv):
    import ml_dtypes

    bf16 = ml_dtypes.bfloat16
    xT = np.ascontiguousarray(x.transpose(0, 2, 1)).astype(bf16)  # [B, E, S]
    wt = np.concatenate(
        [Wk.T, Wq.T, Wk.T, Wv.T, Wv.T], axis=1
    ).astype(bf16)  # [E, 320]
    in_maps = []
    for c in range(N_CORES):
        b, h = divmod(c, 2)
        in_maps.append(
            {
                "xt_q": np.ascontiguousarray(xT[b, :, h * HALF : (h + 1) * HALF]),
                "xt_o": np.ascontiguousarray(
                    xT[b, :, (1 - h) * HALF : (2 - h) * HALF]
                ),
                "wt": wt,
            }
        )
    return in_maps


def _run(x, Wq, Wk, Wv, trace=False):
    from concourse.bass_utils import run_bass_kernel_spmd

    nc = _build()
    in_maps = _make_in_maps(x, Wq, Wk, Wv)
    res = run_bass_kernel_spmd(
        nc, in_maps, core_ids=list(range(N_CORES)), trace=trace
    )
    out = np.empty((B, S, D), dtype=np.float32)
    for c in range(N_CORES):
        b, h = divmod(c, 2)
        att = res.results[c]["out"]  # [65, HALF]: attn^T rows + denom row
        out[b, h * HALF : (h + 1) * HALF, :] = (att[0:D] / att[D : D + 1]).T
    return out, res


def kernel(x, Wq, Wk, Wv):
    out, _ = _run(
        np.asarray(x, dtype=np.float32),
        np.asarray(Wq, dtype=np.float32),
        np.asarray(Wk, dtype=np.float32),
        np.asarray(Wv, dtype=np.float32),
    )
    return out
